# revision 31
# baseline (speedup 1.0000x reference)
"""DigitCapsule (dynamic routing) Trainium2 Bass kernel — v4.

Problem: x (128,1152,8) f32, W (1,1152,10,16,8) f32 ->
  u_hat[b,r,o,do] = sum_di W[r,o,do,di] x[b,r,di]
  3 routing iterations (softmax over routes r, squash), output v (128,10,16,1).

Sharding: data-parallel over batch, 16 samples per core, W replicated.

Per-core layout (partition p = 16*j + b, j = r mod 8, b = batch-in-core):
  u[p, cc, do, o] = u_hat[b, 8*cc+j, o, do]   (fp16, 144 x 16 x 10 free)

Key structure:
  - xd (block-diag x stationary) built on host incl. zeros -> plain DMA
    (input DMA 5.4 MB total; the DMA stream paces the production phase).
  - u produced by 144 matmuls; PSUM->SBUF eviction alternates DVE/Act.
  - s0 = sum_r u via the delta-matrix chain (d16) interleaved into the
    production stream with a 2-batch lag (PE is in-order).
  - squash is elementwise: v = s*|s|/(1+s^2)  (mag_sq in the reference is
    over the trailing singleton axis).  Only Exp/Abs/Square activation
    functions are used -> a single LoadActFuncSet.
  - agreement premul+tree all-fp16 (2x DVE mode); Pool runs group 0's
    premul+tree and the per-group softmax partial sums.
  - exp per group on Act, overlapped with the agreement.
  - s-chain accumulation runs groups in order [1..5, 0] so the slow Pool
    group is consumed last; dummy matmuls pre-warm the PE p-state during
    the softmax window.
"""

import numpy as np

import concourse.bacc as bacc
import concourse.bass as bass
import concourse.tile as tile
from concourse import mybir
from concourse.bass_utils import run_bass_kernel_spmd

B, R, O, DO, DI = 128, 1152, 10, 16, 8
NCORES = 8
BC = B // NCORES          # 16 samples per core
J = 8                     # routes per matmul group
CC = R // J               # 144 matmul groups
OD = O * DO               # 160
F16 = mybir.dt.float16
F32 = mybir.dt.float32
AF = mybir.ActivationFunctionType
ALU = mybir.AluOpType

PROD_BATCH = 2            # cc per production psum batch (1 bank each)
TREE_BATCH = 24           # cc per premult/tree batch
NG = CC // TREE_BATCH     # 6 groups
POOL_GROUP = 0            # premul/tree group owned by the Pool engine
N_WARM = 7                # PE warm-up dummy matmuls per routing iteration


def _tl(pool, shape, tag):
    tile_h = pool.tile(shape, F16, tag=tag, name=tag)
    return tile_h


def _squash_elem(nc, pool, s_ps, v_out, scale, tag):
    """v_out = squash(s_ps * scale) elementwise: v = k2*s*|s| / (1 + (k*s)^2)."""
    P = s_ps.shape[0]
    q = pool.tile([P, DO, O], F32, tag=tag + "q")
    ab = pool.tile([P, DO, O], F32, tag=tag + "a")
    d = pool.tile([P, DO, O], F32, tag=tag + "d")
    p1 = pool.tile([P, DO, O], F32, tag=tag + "p")
    nc.scalar.activation(q[:], s_ps[:], AF.Square, scale=float(scale))
    nc.scalar.activation(ab[:], s_ps[:], AF.Abs, scale=float(scale * scale))
    nc.vector.tensor_scalar_add(d[:], q[:], 1.0)
    nc.vector.reciprocal(d[:], d[:])
    nc.vector.tensor_mul(p1[:], s_ps[:], ab[:])
    nc.vector.tensor_mul(v_out[:], p1[:], d[:])


def build_nc():
    nc = bacc.Bacc("TRN2", debug=False)
    wt_d = nc.dram_tensor("wt", [64, CC, DO, O], F16, kind="ExternalInput")
    xd_d = nc.dram_tensor("xd", [64, CC, 128], F16, kind="ExternalInput")
    d16_d = nc.dram_tensor("d16", [128, 128], F16, kind="ExternalInput")
    d32_d = nc.dram_tensor("d32", [128, 128], F32, kind="ExternalInput")
    dout_d = nc.dram_tensor("dout", [128, BC], F16, kind="ExternalInput")
    out_d = nc.dram_tensor("out", [BC, O, DO], F32, kind="ExternalOutput")

    with tile.TileContext(nc) as tc:
        with (
            tc.tile_pool(name="const", bufs=1) as const,
            tc.tile_pool(name="prod", bufs=1) as prod,
            tc.tile_pool(name="main", bufs=1) as main,
            tc.tile_pool(name="sq", bufs=1) as sq,
            tc.tile_pool(name="tp", bufs=3) as tp,
            tc.tile_pool(name="l1p", bufs=2) as l1p,
            tc.tile_pool(name="l2p", bufs=2) as l2p,
            tc.tile_pool(name="l3p", bufs=2) as l3p,
            tc.tile_pool(name="l4p", bufs=2) as l4p,
            tc.tile_pool(name="pb", bufs=1) as pb,
            tc.tile_pool(name="pp", bufs=3, space=bass.MemorySpace.PSUM) as pp,
            tc.tile_pool(name="pss", bufs=1, space=bass.MemorySpace.PSUM) as pss,
            tc.tile_pool(name="psd", bufs=1, space=bass.MemorySpace.PSUM) as psd,
        ):
            d16 = const.tile([128, 128], F16)
            d32 = const.tile([128, 128], F32)
            dout = const.tile([128, BC], F16)
            nc.sync.dma_start(d16[:], d16_d[:])
            nc.sync.dma_start(d32[:], d32_d[:])
            nc.sync.dma_start(dout[:], dout_d[:])

            NCH = 8
            cch = CC // NCH
            xd_t, wt_t = [None] * NCH, [None] * NCH

            def fetch_chunk(ch):
                sl = slice(ch * cch, (ch + 1) * cch)
                xd_c = prod.tile([64, cch, 128], F16, tag=f"xd{ch}", name="xd_c")
                wt_c = prod.tile([64, cch, DO, O], F16, tag=f"wt{ch}", name="wt_c")
                nc.sync.dma_start(xd_c[:], xd_d[:, sl, :])
                nc.sync.dma_start(wt_c[:], wt_d[:, sl, :, :])
                xd_t[ch] = xd_c
                wt_t[ch] = wt_c

            fetch_chunk(0)
            fetch_chunk(1)

            u = main.tile([128, CC, DO, O], F16)

            # ---- produce u_hat; s0 chain follows two batches behind ----
            s0_ps = pss.tile([128, DO, O], F32, tag="s")
            nb = CC // PROD_BATCH
            bpc = cch // PROD_BATCH       # batches per DMA chunk
            for g in range(nb + 2):
                if g < nb:
                    # prefetch two chunks ahead of consumption
                    if g % bpc == 0 and g // bpc + 2 < NCH:
                        fetch_chunk(g // bpc + 2)
                    ps = pp.tile([128, PROD_BATCH, 512], F32, tag="pp")
                    for i in range(PROD_BATCH):
                        cc = g * PROD_BATCH + i
                        ch, ci = cc // cch, cc % cch
                        nc.tensor.matmul(
                            ps[:, i, 0:OD], xd_t[ch][:, ci, :],
                            wt_t[ch][:, ci, :, :],
                            start=True, stop=True,
                        )
                if g >= 2:
                    for i in range(PROD_BATCH):
                        cc = (g - 2) * PROD_BATCH + i
                        nc.tensor.matmul(
                            s0_ps[:], d16[:], u[:, cc, :, :],
                            start=(cc == 0), stop=(cc == CC - 1),
                        )
                if g < nb:
                    sl = slice(g * PROD_BATCH, (g + 1) * PROD_BATCH)
                    src = ps[:, :, 0:OD].rearrange(
                        "p c (do o) -> p c do o", do=DO)
                    if g % 2 == 0:
                        nc.scalar.copy(u[:, sl, :, :], src)
                    else:
                        nc.vector.tensor_copy(u[:, sl, :, :], src)

            # ---- iter 0: v0 = squash(s0 / R) (already broadcast) ----
            v = main.tile([128, DO, O], F16)
            _squash_elem(nc, sq, s0_ps, v, 1.0 / R, tag="sv")

            b_ij = main.tile([128, CC, O], F32)
            e = main.tile([128, CC, O], F32)
            e_r = main.tile([128, O], F32)
            inv = main.tile([128, O], F32)
            c16 = main.tile([128, CC, O], F16)

            for it in (1, 2):
                final = it == 2
                # ---- agreement: b_ij (+)= sum_do u * v ----
                l3_last = None
                for g in range(NG):
                    sl = slice(g * TREE_BATCH, (g + 1) * TREE_BATCH)
                    pool_g = g == POOL_GROUP
                    eng = nc.gpsimd if pool_g else nc.vector
                    sfx = "P" if pool_g else ""
                    if pool_g or g == 1:
                        t = pb.tile([128, TREE_BATCH, DO, O], F16,
                                    tag=f"t{g}P", name="t")
                    else:
                        t = tp.tile([128, TREE_BATCH, DO, O], F16, tag="t")
                    v_b = v[:].unsqueeze(1).broadcast_to((128, TREE_BATCH, DO, O))
                    eng.tensor_mul(t[:], u[:, sl, :, :], v_b)
                    # group 1's first tree level also goes to Pool (balance)
                    l1_eng = nc.gpsimd if (pool_g or g == 1) else nc.vector
                    l1 = _tl(pb if (pool_g or g == 1) else l1p,
                             [128, TREE_BATCH, 8, O],
                             "l1P" if (pool_g or g == 1) else "l1")
                    l1_eng.tensor_add(l1[:], t[:, :, 0:8, :], t[:, :, 8:16, :])
                    l2 = _tl(pb if pool_g else l2p, [128, TREE_BATCH, 4, O], "l2" + sfx)
                    eng.tensor_add(l2[:], l1[:, :, 0:4, :], l1[:, :, 4:8, :])
                    l3 = _tl(pb if pool_g else l3p, [128, TREE_BATCH, 2, O], "l3" + sfx)
                    eng.tensor_add(l3[:], l2[:, :, 0:2, :], l2[:, :, 2:4, :])
                    if it == 1:
                        eng.tensor_add(
                            b_ij[:, sl, :], l3[:, :, 0, :], l3[:, :, 1, :])
                    else:
                        a4 = _tl(pb if pool_g else l4p, [128, TREE_BATCH, O], "l4" + sfx)
                        eng.tensor_add(a4[:], l3[:, :, 0, :], l3[:, :, 1, :])
                        eng.tensor_add(b_ij[:, sl, :], b_ij[:, sl, :], a4[:])
                    if not pool_g:
                        l3_last = l3
                    # exp of this group overlaps the next group's tree (Act)
                    nc.scalar.activation(e[:, sl, :], b_ij[:, sl, :], AF.Exp)

                # ---- softmax denominator ----
                e_perm = e[:].transpose((0, 2, 1))
                nc.vector.reduce_sum(e_r[:], e_perm, axis=mybir.AxisListType.X)
                # PE p-state warm-up during the softmax window
                warm = psd.tile([128, 512], F32, tag="den")
                for w in range(N_WARM):
                    nc.tensor.matmul(
                        warm[:, 0:480],
                        d16[:], l3_last[:].rearrange("p c x o -> p (c x o)"),
                        start=True, stop=True)
                den = psd.tile([128, 512], F32, tag="den")
                nc.tensor.matmul(den[:, 0:O], d32[:], e_r[:], start=True, stop=True)
                nc.vector.reciprocal(inv[:], den[:, 0:O])

                # ---- s = sum_r c * u  (Pool group last in the psum chain) ----
                sp_p = BC if final else 128
                lhs = dout if final else d16
                s_ps2 = pss.tile([sp_p, DO, O], F32, tag="s")
                inv_b = inv[:].unsqueeze(1).broadcast_to((128, TREE_BATCH, O))
                order = [POOL_GROUP] + [g for g in range(NG) if g != POOL_GROUP]
                t_tiles = {}
                for k, g in enumerate(order):
                    sl = slice(g * TREE_BATCH, (g + 1) * TREE_BATCH)
                    pool_g = g == POOL_GROUP
                    eng = nc.gpsimd if pool_g else nc.vector
                    # c16 always on DVE: it is fast there and would gate
                    # the Pool premul start otherwise
                    nc.vector.tensor_mul(c16[:, sl, :], e[:, sl, :], inv_b)
                    if pool_g:
                        t = pb.tile([128, TREE_BATCH, DO, O], F16, tag="t0P")
                    else:
                        t = tp.tile([128, TREE_BATCH, DO, O], F16, tag="t")
                    c_b = c16[:, sl, :].unsqueeze(2).broadcast_to(
                        (128, TREE_BATCH, DO, O))
                    eng.tensor_mul(t[:], u[:, sl, :, :], c_b)
                    t_tiles[g] = t
                    if pool_g:
                        continue
                    first = k == 1
                    for i in range(TREE_BATCH):
                        nc.tensor.matmul(
                            s_ps2[:], lhs[:, :sp_p], t[:, i, :, :],
                            start=(first and i == 0), stop=False,
                        )
                tpg = t_tiles[POOL_GROUP]
                for i in range(TREE_BATCH):
                    nc.tensor.matmul(
                        s_ps2[:], lhs[:, :sp_p], tpg[:, i, :, :],
                        start=False, stop=(i == TREE_BATCH - 1),
                    )
                if not final:
                    _squash_elem(nc, sq, s_ps2, v, 1.0, tag="sv")
                else:
                    v2 = main.tile([BC, DO, O], F32)
                    _squash_elem(nc, sq, s_ps2, v2, 1.0, tag="sf")
                    v2p = main.tile([BC, O, DO], F32)
                    nc.vector.tensor_copy(v2p[:], v2[:].transpose((0, 2, 1)))
                    nc.sync.dma_start(out_d[:], v2p[:])

    nc.compile()
    return nc


_CACHE = {}


def _get_nc():
    if "nc" not in _CACHE:
        _CACHE["nc"] = build_nc()
    return _CACHE["nc"]


def _prep_const():
    if "const" not in _CACHE:
        p = np.arange(128)
        d16 = (p[:, None] % 16 == p[None, :] % 16).astype(np.float16)
        d32 = d16.astype(np.float32)
        dout = (p[:, None] % 16 == np.arange(BC)[None, :]).astype(np.float16)
        _CACHE["const"] = (d16, d32, dout)
    return _CACHE["const"]


def _prep_w(W):
    W5 = np.ascontiguousarray(W.reshape(R, O, DO, DI))
    # wt[8j+di, cc, do, o] = W[8cc+j, o, do, di]
    wt = np.ascontiguousarray(
        W5.reshape(CC, J, O, DO, DI).transpose(1, 4, 0, 3, 2)
    ).reshape(64, CC, DO, O).astype(np.float16)
    return wt


def kernel(x: np.ndarray, W: np.ndarray) -> np.ndarray:
    x = np.asarray(x, dtype=np.float32)
    W = np.asarray(W, dtype=np.float32)
    nc = _get_nc()
    d16, d32, dout = _prep_const()
    wt = _prep_w(W)
    in_maps = []
    for q in range(NCORES):
        xq = x[BC * q : BC * (q + 1)]           # [16, 1152, 8]
        # xd[8j+di, cc, 16j'+b] = x[b, 8cc+j, di] * (j == j')
        xf = xq.reshape(BC, CC, J, DI).transpose(2, 3, 1, 0)  # [j, di, cc, b]
        xd = np.zeros((J, DI, CC, J, BC), dtype=np.float16)
        for j in range(J):
            xd[j, :, :, j, :] = xf[j]
        xd = np.ascontiguousarray(xd).reshape(64, CC, 128)
        in_maps.append({
            "wt": wt, "xd": xd, "d16": d16, "d32": d32, "dout": dout,
        })
    res = run_bass_kernel_spmd(nc, in_maps, core_ids=list(range(NCORES)))
    out = np.concatenate([res.results[q]["out"] for q in range(NCORES)], axis=0)
    return out.reshape(B, O, DO, 1).astype(np.float32)


# revision 32
# speedup vs baseline: 1.1619x; 1.1619x over previous
"""DigitCapsule (dynamic routing) Trainium2 Bass kernel — v4.

Problem: x (128,1152,8) f32, W (1,1152,10,16,8) f32 ->
  u_hat[b,r,o,do] = sum_di W[r,o,do,di] x[b,r,di]
  3 routing iterations (softmax over routes r, squash), output v (128,10,16,1).

Sharding: data-parallel over batch, 16 samples per core, W replicated.

Per-core layout (partition p = 16*j + b, j = r mod 8, b = batch-in-core):
  u[p, cc, do, o] = u_hat[b, 8*cc+j, o, do]   (fp16, 144 x 16 x 10 free)

Key structure:
  - xd (block-diag x stationary) built on host incl. zeros -> plain DMA
    (input DMA 5.4 MB total; the DMA stream paces the production phase).
  - u produced by 144 matmuls; PSUM->SBUF eviction alternates DVE/Act.
  - s0 = sum_r u via the delta-matrix chain (d16) interleaved into the
    production stream with a 2-batch lag (PE is in-order).
  - squash is elementwise: v = s*|s|/(1+s^2)  (mag_sq in the reference is
    over the trailing singleton axis).  Only Exp/Abs/Square activation
    functions are used -> a single LoadActFuncSet.
  - agreement premul+tree all-fp16 (2x DVE mode); Pool runs group 0's
    premul+tree and the per-group softmax partial sums.
  - exp per group on Act, overlapped with the agreement.
  - s-chain accumulation runs groups in order [1..5, 0] so the slow Pool
    group is consumed last; dummy matmuls pre-warm the PE p-state during
    the softmax window.
"""

import numpy as np

import concourse.bacc as bacc
import concourse.bass as bass
import concourse.tile as tile
from concourse import mybir
from concourse.bass_utils import run_bass_kernel_spmd

B, R, O, DO, DI = 128, 1152, 10, 16, 8
NCORES = 8
BC = B // NCORES          # 16 samples per core
J = 8                     # routes per matmul group
CC = R // J               # 144 matmul groups
OD = O * DO               # 160
F16 = mybir.dt.float16
F32 = mybir.dt.float32
AF = mybir.ActivationFunctionType
ALU = mybir.AluOpType

PROD_BATCH = 2            # cc per production psum batch (1 bank each)
TREE_BATCH = 24           # cc per premult/tree batch
NG = CC // TREE_BATCH     # 6 groups
POOL_GROUP = 0            # premul/tree group owned by the Pool engine
N_WARM = 7                # PE warm-up dummy matmuls per routing iteration


def _tl(pool, shape, tag):
    tile_h = pool.tile(shape, F16, tag=tag, name=tag)
    return tile_h


def _squash_elem(nc, pool, s_ps, v_out, scale, tag):
    """v_out = squash(s_ps * scale) elementwise: v = k2*s*|s| / (1 + (k*s)^2)."""
    P = s_ps.shape[0]
    q = pool.tile([P, DO, O], F32, tag=tag + "q")
    ab = pool.tile([P, DO, O], F32, tag=tag + "a")
    d = pool.tile([P, DO, O], F32, tag=tag + "d")
    p1 = pool.tile([P, DO, O], F32, tag=tag + "p")
    nc.scalar.activation(q[:], s_ps[:], AF.Square, scale=float(scale))
    nc.scalar.activation(ab[:], s_ps[:], AF.Abs, scale=float(scale * scale))
    nc.vector.tensor_scalar_add(d[:], q[:], 1.0)
    nc.vector.reciprocal(d[:], d[:])
    nc.vector.tensor_mul(p1[:], s_ps[:], ab[:])
    nc.vector.tensor_mul(v_out[:], p1[:], d[:])


def build_nc():
    nc = bacc.Bacc("TRN2", debug=False)
    wt_d = nc.dram_tensor("wt", [64, CC, DO, O], F16, kind="ExternalInput")
    xd_d = nc.dram_tensor("xd", [64, CC, 128], F16, kind="ExternalInput")
    d16_d = nc.dram_tensor("d16", [128, 128], F16, kind="ExternalInput")
    d32_d = nc.dram_tensor("d32", [128, 128], F32, kind="ExternalInput")
    dout_d = nc.dram_tensor("dout", [128, BC], F16, kind="ExternalInput")
    out_d = nc.dram_tensor("out", [BC, O, DO], F32, kind="ExternalOutput")

    with tile.TileContext(nc) as tc:
        with (
            tc.tile_pool(name="const", bufs=1) as const,
            tc.tile_pool(name="prod", bufs=1) as prod,
            tc.tile_pool(name="main", bufs=1) as main,
            tc.tile_pool(name="sq", bufs=1) as sq,
            tc.tile_pool(name="tp", bufs=3) as tp,
            tc.tile_pool(name="l1p", bufs=2) as l1p,
            tc.tile_pool(name="l2p", bufs=2) as l2p,
            tc.tile_pool(name="l3p", bufs=2) as l3p,
            tc.tile_pool(name="l4p", bufs=2) as l4p,
            tc.tile_pool(name="pb", bufs=1) as pb,
            tc.tile_pool(name="pp", bufs=3, space=bass.MemorySpace.PSUM) as pp,
            tc.tile_pool(name="pss", bufs=1, space=bass.MemorySpace.PSUM) as pss,
            tc.tile_pool(name="psd", bufs=1, space=bass.MemorySpace.PSUM) as psd,
        ):
            d16 = const.tile([128, 128], F16)
            d32 = const.tile([128, 128], F32)
            dout = const.tile([128, BC], F16)
            nc.sync.dma_start(d16[:], d16_d[:])
            nc.sync.dma_start(d32[:], d32_d[:])
            nc.sync.dma_start(dout[:], dout_d[:])

            NCH = 8
            cch = CC // NCH
            xd_t, wt_t = [None] * NCH, [None] * NCH

            def fetch_chunk(ch):
                sl = slice(ch * cch, (ch + 1) * cch)
                xd_c = prod.tile([64, cch, 128], F16, tag=f"xd{ch}", name="xd_c")
                wt_c = prod.tile([64, cch, DO, O], F16, tag=f"wt{ch}", name="wt_c")
                nc.sync.dma_start(xd_c[:], xd_d[:, sl, :])
                nc.sync.dma_start(wt_c[:], wt_d[:, sl, :, :])
                xd_t[ch] = xd_c
                wt_t[ch] = wt_c

            fetch_chunk(0)
            fetch_chunk(1)

            u = main.tile([128, CC, DO, O], F16)

            # ---- produce u_hat; s0 chain follows two batches behind ----
            s0_ps = pss.tile([128, DO, O], F32, tag="s")
            nb = CC // PROD_BATCH
            bpc = cch // PROD_BATCH       # batches per DMA chunk
            LAG = 6
            for g in range(nb + LAG):
                if g < nb:
                    # prefetch two chunks ahead of consumption
                    if g % bpc == 0 and g // bpc + 2 < NCH:
                        fetch_chunk(g // bpc + 2)
                    ps = pp.tile([128, PROD_BATCH, 512], F32, tag="pp")
                    for i in range(PROD_BATCH):
                        cc = g * PROD_BATCH + i
                        ch, ci = cc // cch, cc % cch
                        nc.tensor.matmul(
                            ps[:, i, 0:OD], xd_t[ch][:, ci, :],
                            wt_t[ch][:, ci, :, :],
                            start=True, stop=True,
                        )
                if g >= LAG:
                    for i in range(PROD_BATCH):
                        cc = (g - LAG) * PROD_BATCH + i
                        nc.tensor.matmul(
                            s0_ps[:], d16[:], u[:, cc, :, :],
                            start=(cc == 0), stop=(cc == CC - 1),
                        )
                if g < nb:
                    sl = slice(g * PROD_BATCH, (g + 1) * PROD_BATCH)
                    src = ps[:, :, 0:OD].rearrange(
                        "p c (do o) -> p c do o", do=DO)
                    if g % 2 == 0:
                        nc.scalar.copy(u[:, sl, :, :], src)
                    else:
                        nc.vector.tensor_copy(u[:, sl, :, :], src)

            # ---- iter 0: v0 = squash(s0 / R) (already broadcast) ----
            v = main.tile([128, DO, O], F16)
            _squash_elem(nc, sq, s0_ps, v, 1.0 / R, tag="sv")

            b_ij = main.tile([128, CC, O], F32)
            e = main.tile([128, CC, O], F32)
            e_r = main.tile([128, O], F32)
            inv = main.tile([128, O], F32)
            c16 = main.tile([128, CC, O], F16)

            for it in (1, 2):
                final = it == 2
                # ---- agreement: b_ij (+)= sum_do u * v ----
                l3_last = None
                for g in range(NG):
                    sl = slice(g * TREE_BATCH, (g + 1) * TREE_BATCH)
                    pool_g = g == POOL_GROUP
                    eng = nc.gpsimd if pool_g else nc.vector
                    sfx = "P" if pool_g else ""
                    if pool_g:
                        t = pb.tile([128, TREE_BATCH, DO, O], F16,
                                    tag="t0P", name="t")
                    else:
                        t = tp.tile([128, TREE_BATCH, DO, O], F16, tag="t")
                    v_b = v[:].unsqueeze(1).broadcast_to((128, TREE_BATCH, DO, O))
                    eng.tensor_mul(t[:], u[:, sl, :, :], v_b)
                    l1 = _tl(pb if pool_g else l1p, [128, TREE_BATCH, 8, O], "l1" + sfx)
                    eng.tensor_add(l1[:], t[:, :, 0:8, :], t[:, :, 8:16, :])
                    l2 = _tl(pb if pool_g else l2p, [128, TREE_BATCH, 4, O], "l2" + sfx)
                    eng.tensor_add(l2[:], l1[:, :, 0:4, :], l1[:, :, 4:8, :])
                    l3 = _tl(pb if pool_g else l3p, [128, TREE_BATCH, 2, O], "l3" + sfx)
                    eng.tensor_add(l3[:], l2[:, :, 0:2, :], l2[:, :, 2:4, :])
                    if it == 1:
                        eng.tensor_add(
                            b_ij[:, sl, :], l3[:, :, 0, :], l3[:, :, 1, :])
                    else:
                        a4 = _tl(pb if pool_g else l4p, [128, TREE_BATCH, O], "l4" + sfx)
                        eng.tensor_add(a4[:], l3[:, :, 0, :], l3[:, :, 1, :])
                        eng.tensor_add(b_ij[:, sl, :], b_ij[:, sl, :], a4[:])
                    if not pool_g:
                        l3_last = l3
                    # exp of this group overlaps the next group's tree (Act)
                    nc.scalar.activation(e[:, sl, :], b_ij[:, sl, :], AF.Exp)

                # ---- softmax denominator ----
                e_perm = e[:].transpose((0, 2, 1))
                nc.vector.reduce_sum(e_r[:], e_perm, axis=mybir.AxisListType.X)
                # PE p-state warm-up during the softmax window
                warm = psd.tile([128, 512], F32, tag="den")
                for w in range(N_WARM):
                    nc.tensor.matmul(
                        warm[:, 0:480],
                        d16[:], l3_last[:].rearrange("p c x o -> p (c x o)"),
                        start=True, stop=True)
                den = psd.tile([128, 512], F32, tag="den")
                nc.tensor.matmul(den[:, 0:O], d32[:], e_r[:], start=True, stop=True)
                nc.vector.reciprocal(inv[:], den[:, 0:O])

                # ---- s = sum_r c * u  (Pool group last in the psum chain) ----
                sp_p = BC if final else 128
                lhs = dout if final else d16
                s_ps2 = pss.tile([sp_p, DO, O], F32, tag="s")
                inv_b = inv[:].unsqueeze(1).broadcast_to((128, TREE_BATCH, O))
                order = [POOL_GROUP] + [g for g in range(NG) if g != POOL_GROUP]
                t_tiles = {}
                for k, g in enumerate(order):
                    sl = slice(g * TREE_BATCH, (g + 1) * TREE_BATCH)
                    pool_g = g == POOL_GROUP
                    eng = nc.gpsimd if pool_g else nc.vector
                    # c16 always on DVE: it is fast there and would gate
                    # the Pool premul start otherwise
                    nc.vector.tensor_mul(c16[:, sl, :], e[:, sl, :], inv_b)
                    if pool_g:
                        t = pb.tile([128, TREE_BATCH, DO, O], F16, tag="t0P")
                    else:
                        t = tp.tile([128, TREE_BATCH, DO, O], F16, tag="t")
                    c_b = c16[:, sl, :].unsqueeze(2).broadcast_to(
                        (128, TREE_BATCH, DO, O))
                    eng.tensor_mul(t[:], u[:, sl, :, :], c_b)
                    t_tiles[g] = t
                    if pool_g:
                        continue
                    first = k == 1
                    for i in range(TREE_BATCH):
                        nc.tensor.matmul(
                            s_ps2[:], lhs[:, :sp_p], t[:, i, :, :],
                            start=(first and i == 0), stop=False,
                        )
                tpg = t_tiles[POOL_GROUP]
                for i in range(TREE_BATCH):
                    nc.tensor.matmul(
                        s_ps2[:], lhs[:, :sp_p], tpg[:, i, :, :],
                        start=False, stop=(i == TREE_BATCH - 1),
                    )
                if not final:
                    _squash_elem(nc, sq, s_ps2, v, 1.0, tag="sv")
                else:
                    v2 = main.tile([BC, DO, O], F32)
                    _squash_elem(nc, sq, s_ps2, v2, 1.0, tag="sf")
                    v2p = main.tile([BC, O, DO], F32)
                    nc.vector.tensor_copy(v2p[:], v2[:].transpose((0, 2, 1)))
                    nc.sync.dma_start(out_d[:], v2p[:])

    nc.compile()
    return nc


_CACHE = {}


def _get_nc():
    if "nc" not in _CACHE:
        _CACHE["nc"] = build_nc()
    return _CACHE["nc"]


def _prep_const():
    if "const" not in _CACHE:
        p = np.arange(128)
        d16 = (p[:, None] % 16 == p[None, :] % 16).astype(np.float16)
        d32 = d16.astype(np.float32)
        dout = (p[:, None] % 16 == np.arange(BC)[None, :]).astype(np.float16)
        _CACHE["const"] = (d16, d32, dout)
    return _CACHE["const"]


def _prep_w(W):
    W5 = np.ascontiguousarray(W.reshape(R, O, DO, DI))
    # wt[8j+di, cc, do, o] = W[8cc+j, o, do, di]
    wt = np.ascontiguousarray(
        W5.reshape(CC, J, O, DO, DI).transpose(1, 4, 0, 3, 2)
    ).reshape(64, CC, DO, O).astype(np.float16)
    return wt


def kernel(x: np.ndarray, W: np.ndarray) -> np.ndarray:
    x = np.asarray(x, dtype=np.float32)
    W = np.asarray(W, dtype=np.float32)
    nc = _get_nc()
    d16, d32, dout = _prep_const()
    wt = _prep_w(W)
    in_maps = []
    for q in range(NCORES):
        xq = x[BC * q : BC * (q + 1)]           # [16, 1152, 8]
        # xd[8j+di, cc, 16j'+b] = x[b, 8cc+j, di] * (j == j')
        xf = xq.reshape(BC, CC, J, DI).transpose(2, 3, 1, 0)  # [j, di, cc, b]
        xd = np.zeros((J, DI, CC, J, BC), dtype=np.float16)
        for j in range(J):
            xd[j, :, :, j, :] = xf[j]
        xd = np.ascontiguousarray(xd).reshape(64, CC, 128)
        in_maps.append({
            "wt": wt, "xd": xd, "d16": d16, "d32": d32, "dout": dout,
        })
    res = run_bass_kernel_spmd(nc, in_maps, core_ids=list(range(NCORES)))
    out = np.concatenate([res.results[q]["out"] for q in range(NCORES)], axis=0)
    return out.reshape(B, O, DO, 1).astype(np.float32)
